# revision 8
# baseline (speedup 1.0000x reference)
"""Trainium2 Bass kernel for nn_KNNModule_2946347565933.

Effective computation (batch/KNN collapse to a residual delta-MLP; `batch` is
unused by the reference):
    w = lrelu(bn(weights @ ri_W0)); w = lrelu(bn(w @ ri_W1))
    for l in 0..3:  h = lrelu(bn(w @ dW0[l])); d = h @ dW1[l] + db1[l]
                    pos += d[:, :2]; w += d[:, 2:]
    h = lrelu(bn(w @ ro_W0)); w_out = h @ ro_W1 + ro_b1
    return pos, w_out

Strategy (8 cores, data-parallel over N=400000):
 - channels-on-partitions layout: per-core residual stream STR [128, 50000]
   fp16 SBUF-resident; a second buffer PRE [128, 50000] fp16 holds the next
   BN layer's pre-activations so they are never recomputed on the PE.
 - per phase and tile: one Lrelu activation on ScalarE (normalize+act fused,
   per-partition scale/bias), matmuls on PE, PSUM->SBUF preact stores on
   GpSimd, bn_stats on fp16 on DVE, residual adds on DVE.
 - 7 BN sync points. Layer-1 stats are exact from the host (2x2 second-moment
   of `weights`). The other 6: local bn_aggr -> [mean, E[x^2]] -> AllReduce
   (add) of [128,2] f32 across the 8 cores -> s,t.
 - last block's residual add is folded into the readout preact:
   p_ro = roW0^T x3 + (dW1w3 @ roW0)^T h3, so x4 is never materialized.
 - dpos/wout are DMA'd straight from PSUM (f32); db1/ro_b1 and the final pos
   accumulation are applied on host (pos never touches the device).
"""
import os
import sys

sys.path.insert(0, "/opt/trn_rl_repo")

from contextlib import ExitStack

import numpy as np

import concourse.bass as bass
import concourse.bacc as bacc
import concourse.mybir as mybir
import concourse.tile as tile
from concourse.bass_utils import run_bass_kernel_spmd

F32 = mybir.dt.float32
F16 = mybir.dt.float16  # fp16: same PE rate as bf16, 8x finer mantissa

NCORES = 8
N, D, C_IN, H, C_OUT, L = 400000, 2, 2, 128, 2, 4
R = N // NCORES          # rows per core
TF = 500                 # tile free size (rows per tile)
T = R // TF              # tiles per pass
EPS = 1e-5
SLOPE = 0.01

_cache = {}


def _install_trace_hook():
    """Recreate the missing antenv.axon_hooks NTFF-profile hook via ctypes so
    run_bass_kernel_spmd(trace=True) can capture device profiles under axon."""
    import types

    if "antenv.axon_hooks" not in sys.modules:
        mod = types.ModuleType("antenv.axon_hooks")
        mod._h = None
        mod.set_axon_ntff_profile_hook = lambda h: setattr(mod, "_h", h)
        mod.get_axon_ntff_profile_hook = lambda: mod._h
        sys.modules["antenv.axon_hooks"] = mod
        import antenv

        antenv.axon_hooks = mod
    from antenv.axon_hooks import (
        get_axon_ntff_profile_hook,
        set_axon_ntff_profile_hook,
    )

    if get_axon_ntff_profile_hook() is None:
        if "/root/.axon_site" not in sys.path:
            sys.path.insert(0, "/root/.axon_site")
        from trn_agent_boot.trn_boot import _ntff_profile_via_ctypes

        set_axon_ntff_profile_hook(
            _ntff_profile_via_ctypes("/opt/axon/libaxon_pjrt.so"))
    import concourse.bass_utils as bu

    bu.upload_artifacts = lambda tmpdir: "local://" + tmpdir


def _build():
    nc = bacc.Bacc("TRN2", target_bir_lowering=False, debug=False,
                   num_devices=NCORES)
    # ---- I/O ----
    w0t_d = nc.dram_tensor("w0t", [C_IN, R], F16, kind="ExternalInput")
    riW0_d = nc.dram_tensor("riW0", [C_IN, H], F16, kind="ExternalInput")
    riW1_d = nc.dram_tensor("riW1", [H, H], F16, kind="ExternalInput")
    dW0_d = nc.dram_tensor("dW0", [L, H, H], F16, kind="ExternalInput")
    # dW1w only needed for blocks 0..2; block 3 uses the composite M3
    dW1w_d = nc.dram_tensor("dW1w", [L - 1, H, H], F16, kind="ExternalInput")
    dW1p_d = nc.dram_tensor("dW1p", [L, H, D], F16, kind="ExternalInput")
    roW0_d = nc.dram_tensor("roW0", [H, H], F16, kind="ExternalInput")
    m3_d = nc.dram_tensor("m3", [H, H], F16, kind="ExternalInput")
    roW1_d = nc.dram_tensor("roW1", [H, C_OUT], F16, kind="ExternalInput")
    # per-partition BN params: col k = BN layer k+2 (layers 2..7)
    g_d = nc.dram_tensor("gT", [H, 6], F32, kind="ExternalInput")
    be_d = nc.dram_tensor("beT", [H, 6], F32, kind="ExternalInput")
    s1t1_d = nc.dram_tensor("s1t1", [H, 2], F32, kind="ExternalInput")

    dpos_d = nc.dram_tensor("dpos", [L, D, R], F16, kind="ExternalOutput")
    wout_d = nc.dram_tensor("wout", [C_OUT, R], F16, kind="ExternalOutput")

    with tile.TileContext(nc) as tc, ExitStack() as ctx:
        P = H
        sb = ctx.enter_context(tc.tile_pool(name="sb", bufs=1))
        hpool = ctx.enter_context(tc.tile_pool(name="hp", bufs=2))
        dpst = ctx.enter_context(tc.tile_pool(name="dpst", bufs=2))
        w0pool = ctx.enter_context(tc.tile_pool(name="w0p", bufs=2))
        recp = ctx.enter_context(tc.tile_pool(name="recp", bufs=1))
        stp = ctx.enter_context(tc.tile_pool(name="stp", bufs=4))
        smalls = ctx.enter_context(tc.tile_pool(name="smalls", bufs=2))
        pa = ctx.enter_context(tc.tile_pool(name="pa", bufs=2, space="PSUM"))
        pn = ctx.enter_context(tc.tile_pool(name="pn", bufs=2, space="PSUM"))
        pu = ctx.enter_context(tc.tile_pool(name="pu", bufs=2, space="PSUM"))
        pd = ctx.enter_context(tc.tile_pool(name="pd", bufs=2, space="PSUM"))
        dram = ctx.enter_context(tc.tile_pool(name="dram", bufs=2, space="DRAM"))

        # ---- resident SBUF tensors ----
        STR = sb.tile([P, R], F16, tag="STR")
        PRE = sb.tile([P, R], F16, tag="PRE")
        riW0 = sb.tile([C_IN, H], F16, tag="riW0")
        riW1 = sb.tile([H, H], F16, tag="riW1")
        dW0 = [sb.tile([H, H], F16, tag=f"dW0_{l}", name=f"dW0_{l}")
               for l in range(L)]
        dW1w = [sb.tile([H, H], F16, tag=f"dW1w_{l}", name=f"dW1w_{l}")
                for l in range(L - 1)]
        dW1p = [sb.tile([H, D], F16, tag=f"dW1p_{l}", name=f"dW1p_{l}")
                for l in range(L)]
        roW0 = sb.tile([H, H], F16, tag="roW0")
        M3 = sb.tile([H, H], F16, tag="M3")
        roW1 = sb.tile([H, C_OUT], F16, tag="roW1")
        gT = sb.tile([H, 6], F32, tag="gT")
        beT = sb.tile([H, 6], F32, tag="beT")
        s1t1 = sb.tile([H, 2], F32, tag="s1t1")
        epst = sb.tile([H, 1], F32, tag="epst")

        nc.sync.dma_start(out=riW0, in_=riW0_d.ap())
        nc.sync.dma_start(out=riW1, in_=riW1_d.ap())
        for l in range(L):
            nc.sync.dma_start(out=dW0[l], in_=dW0_d.ap()[l])
            nc.sync.dma_start(out=dW1p[l], in_=dW1p_d.ap()[l])
        for l in range(L - 1):
            nc.sync.dma_start(out=dW1w[l], in_=dW1w_d.ap()[l])
        nc.sync.dma_start(out=roW0, in_=roW0_d.ap())
        nc.sync.dma_start(out=M3, in_=m3_d.ap())
        nc.sync.dma_start(out=roW1, in_=roW1_d.ap())
        nc.sync.dma_start(out=gT, in_=g_d.ap())
        nc.sync.dma_start(out=beT, in_=be_d.ap())
        nc.sync.dma_start(out=s1t1, in_=s1t1_d.ap())
        nc.vector.memset(epst, EPS)

        def stats_merge(rec, k):
            """rec [P, T, 6] local bn_stats records -> (s, t) for BN layer k+2
            via an AllReduce-add of [mean, E[x^2]] over the 8 cores."""
            mv = smalls.tile([P, 2], F32, tag="mv")
            nc.vector.bn_aggr(out=mv, in_=rec[:])
            cc_in = dram.tile([P, 2], F32, tag="cc_in")
            loc = smalls.tile([P, 2], F32, tag="loc")
            nc.vector.tensor_copy(out=loc[:, 0:1], in_=mv[:, 0:1])
            # E[x^2] = var + mean^2
            nc.vector.scalar_tensor_tensor(
                out=loc[:, 1:2], in0=mv[:, 0:1], scalar=1.0, in1=mv[:, 0:1],
                op0=mybir.AluOpType.mult, op1=mybir.AluOpType.mult)
            nc.vector.tensor_add(out=loc[:, 1:2], in0=loc[:, 1:2],
                                 in1=mv[:, 1:2])
            nc.sync.dma_start(out=cc_in[:], in_=loc[:])
            cc_out = dram.tile([P, 2], F32, tag="cc_out")
            nc.gpsimd.collective_compute(
                "AllReduce", mybir.AluOpType.add,
                replica_groups=[list(range(NCORES))],
                ins=[cc_in.opt()], outs=[cc_out.opt()],
            )
            gsum = smalls.tile([P, 2], F32, tag="gsum")
            nc.sync.dma_start(out=gsum[:], in_=cc_out[:])
            gm = stp.tile([P, 1], F32, tag="gm")
            nc.vector.tensor_scalar_mul(out=gm, in0=gsum[:, 0:1],
                                        scalar1=1.0 / NCORES)
            var = stp.tile([P, 1], F32, tag="var")
            # var = E[x^2] - mean^2 = gsum1/8 - gm^2
            nc.vector.scalar_tensor_tensor(
                out=var, in0=gm, scalar=-1.0, in1=gm,
                op0=mybir.AluOpType.mult, op1=mybir.AluOpType.mult)
            nc.vector.scalar_tensor_tensor(
                out=var, in0=gsum[:, 1:2], scalar=1.0 / NCORES, in1=var,
                op0=mybir.AluOpType.mult, op1=mybir.AluOpType.add)
            s = stp.tile([P, 1], F32, tag="s")
            t = stp.tile([P, 1], F32, tag="t")
            nc.scalar.activation(out=s, in_=var,
                                 func=mybir.ActivationFunctionType.Sqrt,
                                 bias=epst[:], scale=1.0)
            nc.vector.reciprocal(out=s, in_=s)
            nc.vector.tensor_mul(out=s, in0=s, in1=gT[:, k:k + 1])
            nc.vector.tensor_mul(out=t, in0=gm, in1=s)
            nc.vector.tensor_sub(out=t, in0=beT[:, k:k + 1], in1=t)
            return s, t

        ts = bass.ts
        LR = mybir.ActivationFunctionType.Lrelu

        # ---- PH1: x1 = act(riW0^T w0) [host stats]; a2 = riW1^T x1 ----
        rec = recp.tile([P, T, 6], F32, tag="rec")
        for i in range(T):
            w0 = w0pool.tile([C_IN, TF], F16, tag="w0")
            nc.sync.dma_start(out=w0, in_=w0t_d.ap()[:, ts(i, TF)])
            a1 = pa.tile([P, TF], F32, tag="a1")
            nc.tensor.matmul(out=a1[:], lhsT=riW0[:], rhs=w0[:],
                             start=True, stop=True)
            nc.scalar.activation(out=STR[:, ts(i, TF)], in_=a1[:],
                                 func=LR, bias=s1t1[:, 1:2],
                                 scale=s1t1[:, 0:1], alpha=SLOPE)
            a2 = pn.tile([P, TF], F32, tag="a2")
            nc.tensor.matmul(out=a2[:], lhsT=riW1[:], rhs=STR[:, ts(i, TF)],
                             start=True, stop=True)
            if i % 2 == 0:
                nc.scalar.copy(out=PRE[:, ts(i, TF)], in_=a2[:])
            else:
                nc.vector.tensor_copy(out=PRE[:, ts(i, TF)], in_=a2[:])
            nc.vector.bn_stats(out=rec[:, i, :], in_=PRE[:, ts(i, TF)])
        s, t = stats_merge(rec, 0)

        # ---- PH2: x2 = act(PRE); a3 = dW0[0]^T x2 ----
        rec = recp.tile([P, T, 6], F32, tag="rec")
        for i in range(T):
            nc.scalar.activation(out=STR[:, ts(i, TF)], in_=PRE[:, ts(i, TF)],
                                 func=LR, bias=t[:], scale=s[:], alpha=SLOPE)
            a3 = pn.tile([P, TF], F32, tag="a2")
            nc.tensor.matmul(out=a3[:], lhsT=dW0[0][:],
                             rhs=STR[:, ts(i, TF)], start=True, stop=True)
            if i % 2 == 0:
                nc.scalar.copy(out=PRE[:, ts(i, TF)], in_=a3[:])
            else:
                nc.vector.tensor_copy(out=PRE[:, ts(i, TF)], in_=a3[:])
            nc.vector.bn_stats(out=rec[:, i, :], in_=PRE[:, ts(i, TF)])
        s, t = stats_merge(rec, 1)

        # ---- PH3..PH6: blocks ----
        for l in range(L):
            rec = recp.tile([P, T, 6], F32, tag="rec")
            last = l == L - 1
            nxt = roW0 if last else dW0[l + 1]
            for i in range(T):
                h = hpool.tile([P, TF], F16, tag="h")
                nc.scalar.activation(out=h, in_=PRE[:, ts(i, TF)],
                                     func=LR, bias=t[:], scale=s[:],
                                     alpha=SLOPE)
                dp = pd.tile([D, TF], F32, tag="dp")
                nc.tensor.matmul(out=dp[:], lhsT=dW1p[l][:], rhs=h[:],
                                 start=True, stop=True)
                dps = dpst.tile([D, TF], F16, tag="dps")
                if i % 2 == 0:
                    nc.scalar.copy(out=dps, in_=dp[:])
                else:
                    nc.vector.tensor_copy(out=dps, in_=dp[:])
                nc.sync.dma_start(out=dpos_d.ap()[l, :, ts(i, TF)], in_=dps[:])
                an = pn.tile([P, TF], F32, tag="a2")
                if last:
                    # p_ro = roW0^T x3 + (dW1w3 roW0)^T h3; x4 never formed
                    nc.tensor.matmul(out=an[:], lhsT=roW0[:],
                                     rhs=STR[:, ts(i, TF)],
                                     start=True, stop=False)
                    nc.tensor.matmul(out=an[:], lhsT=M3[:], rhs=h[:],
                                     start=False, stop=True)
                else:
                    dw = pu.tile([P, TF], F32, tag="dw")
                    nc.tensor.matmul(out=dw[:], lhsT=dW1w[l][:], rhs=h[:],
                                     start=True, stop=True)
                    nc.vector.tensor_add(out=STR[:, ts(i, TF)],
                                         in0=STR[:, ts(i, TF)], in1=dw[:])
                    nc.tensor.matmul(out=an[:], lhsT=nxt[:],
                                     rhs=STR[:, ts(i, TF)],
                                     start=True, stop=True)
                if i % 2 == 0:
                    nc.scalar.copy(out=PRE[:, ts(i, TF)], in_=an[:])
                else:
                    nc.vector.tensor_copy(out=PRE[:, ts(i, TF)], in_=an[:])
                nc.vector.bn_stats(out=rec[:, i, :], in_=PRE[:, ts(i, TF)])
            s, t = stats_merge(rec, 2 + l)

        # ---- PH7: readout ----
        for i in range(T):
            h = hpool.tile([P, TF], F16, tag="h")
            nc.scalar.activation(out=h, in_=PRE[:, ts(i, TF)],
                                 func=LR, bias=t[:], scale=s[:], alpha=SLOPE)
            o = pd.tile([C_OUT, TF], F32, tag="dp")
            nc.tensor.matmul(out=o[:], lhsT=roW1[:], rhs=h[:],
                             start=True, stop=True)
            os_ = dpst.tile([C_OUT, TF], F16, tag="dps")
            if i % 2 == 0:
                nc.vector.tensor_copy(out=os_, in_=o[:])
            else:
                nc.scalar.copy(out=os_, in_=o[:])
            nc.sync.dma_start(out=wout_d.ap()[:, ts(i, TF)], in_=os_[:])

    nc.compile()
    return nc


def kernel(positions, weights, batch,
           ri_W0, ri_b0, ri_g0, ri_be0, ri_W1, ri_b1, ri_g1, ri_be1,
           dW0, db0, dg0, dbe0, dW1, db1,
           ro_W0, ro_b0, ro_g0, ro_be0, ro_W1, ro_b1):
    positions = np.asarray(positions, np.float32)
    weights = np.asarray(weights, np.float32)

    if "nc" not in _cache:
        _cache["nc"] = _build()
    nc = _cache["nc"]

    bf = lambda x: np.asarray(x, np.float32).astype(np.float16)

    # host: exact L1 BN stats from the 2x2 second moment of `weights`
    # (linear bias ri_b0 cancels inside BN)
    w64 = weights.astype(np.float64)
    m1 = w64.mean(0)                       # [2]
    m2 = (w64.T @ w64) / N                 # [2,2]
    # device computes a1 with fp16-rounded inputs; match those moments
    W0r = bf(ri_W0).astype(np.float64)
    mu1 = m1 @ W0r
    e2 = np.einsum("kc,kl,lc->c", W0r, m2, W0r)
    var1 = e2 - mu1 * mu1
    s1 = np.asarray(ri_g0, np.float64) / np.sqrt(var1 + EPS)
    t1 = np.asarray(ri_be0, np.float64) - mu1 * s1
    s1t1 = np.stack([s1, t1], 1).astype(np.float32)   # [128, 2]

    gT = np.stack([ri_g1, dg0[0], dg0[1], dg0[2], dg0[3], ro_g0], 1)
    beT = np.stack([ri_be1, dbe0[0], dbe0[1], dbe0[2], dbe0[3], ro_be0], 1)

    dW1 = np.asarray(dW1, np.float32)
    dW1w_full = np.ascontiguousarray(dW1[:, :, D:]).astype(np.float64)
    m3 = (dW1w_full[L - 1] @ np.asarray(ro_W0, np.float64))
    shared = dict(
        riW0=bf(ri_W0), riW1=bf(ri_W1),
        dW0=bf(dW0), dW1w=bf(dW1w_full[:L - 1]),
        dW1p=bf(np.ascontiguousarray(dW1[:, :, :D])),
        roW0=bf(ro_W0), m3=bf(m3), roW1=bf(ro_W1),
        gT=np.asarray(gT, np.float32), beT=np.asarray(beT, np.float32),
        s1t1=s1t1,
    )
    in_maps = []
    for c in range(NCORES):
        sl = weights[c * R:(c + 1) * R]
        in_maps.append(dict(shared, w0t=bf(np.ascontiguousarray(sl.T))))

    trace = bool(int(os.environ.get("KERNEL_TRACE", "0")))
    kw = {}
    if trace:
        _install_trace_hook()
        kw["tmpdir"] = os.environ.get("KERNEL_TRACE_DIR") or None
    res = run_bass_kernel_spmd(
        nc, in_maps, core_ids=list(range(NCORES)), trace=trace, **kw,
    )
    _cache["last_results"] = res

    # assemble
    pos = positions.astype(np.float64)
    db1 = np.asarray(db1, np.float64)
    wout = np.empty((N, C_OUT), np.float32)
    dsum = np.zeros((N, D), np.float64)
    for c in range(NCORES):
        r = res.results[c]
        dsum[c * R:(c + 1) * R] += r["dpos"].astype(np.float64).sum(0).T
        wout[c * R:(c + 1) * R] = r["wout"].T
    pos = pos + dsum + db1[:, :D].sum(0)
    wout = (wout.astype(np.float64) + np.asarray(ro_b1, np.float64)).astype(np.float32)
    return pos.astype(np.float32), wout
